# revision 27
# baseline (speedup 1.0000x reference)
"""GAT 3-layer kernel for TRN2, 8 NeuronCores (SPMD).

Strategy:
- Relabel nodes: greedy-balanced A/B table-half split (each dst's in-edges
  split ~evenly between halves, exact-gain refinement) then
  max(dA,dB)-sorted round-robin deal to 8 cores; each core owns NP local
  node slots as TILES tiles of 128.
- Per layer: fused stage A computes [feat | el | er] with one matmul against
  W_ext = [W | W@al | W@ar]; writes a bf16 gather-table shard [NP, R]
  (feat bf16 + el raw-f32 bitcast), AllGather -> full table [8*NP, R].
- Edge aggregation per node tile (padded CSR): dma_gather (int16 idx; table
  in two halves to fit int16), gathers grouped in tile PAIRS (high+low
  degree) to amortize fixed SWDGE overhead; softmax without max-subtract
  (logits are O(1); sentinel rows el=-1e30 -> exp=0; s clamped before
  reciprocal); leaky-relu (Prelu) and exp+sum run on the Scalar engine.
- Stage A of layer l+1 is fused into stage C of layer l (TensorE transpose
  of o, matmul W_ext directly) - no hT DRAM round trip.
- All copy/cast/scalar ops go on ACT: DVE copy/cast/tensor_scalar grab the
  GpSimd-shared SBUF port pair and stall against in-flight gather
  descriptor generation (tensor_tensor/reduce never contend).

kernel(**inputs) takes FULL inputs, returns FULL [N, OUT] output.
"""
import os
import numpy as np
import ml_dtypes

C = 8          # cores
P = 128        # partitions


# ----------------------------------------------------------------- host prep
def _balanced_split(src, dst, N, capA):
    """Greedy A/B split of source nodes s.t. each dst's in-edges divide
    ~evenly between halves, with exact-gain refinement passes."""
    out_deg = np.bincount(src, minlength=N)
    order_s = np.argsort(src, kind="stable")
    dst_by_src = dst[order_s]
    sstart = np.zeros(N + 1, dtype=np.int64)
    np.cumsum(out_deg, out=sstart[1:])
    bal = np.zeros(N, dtype=np.int32)       # dA - dB per dst
    inA = np.zeros(N, dtype=bool)
    szA = szB = 0
    capB = capA
    szmin = N - capA
    for v in np.argsort(-out_deg, kind="stable"):
        ds = dst_by_src[sstart[v]:sstart[v + 1]]
        s = int(bal[ds].sum()) if len(ds) else (szA - szB)
        if (s < 0 or szB >= capB) and szA < capA:
            inA[v] = True
            szA += 1
            if len(ds):
                bal[ds] += 1
        else:
            szB += 1
            if len(ds):
                bal[ds] -= 1
    for _ in range(4):
        flips = 0
        for v in range(N):
            ds = dst_by_src[sstart[v]:sstart[v + 1]]
            if len(ds) == 0:
                continue
            b = bal[ds]
            if inA[v]:
                if np.abs(b).sum() - np.abs(b - 2).sum() > 0 and szA > szmin:
                    inA[v] = False
                    bal[ds] -= 2
                    szA -= 1
                    flips += 1
            else:
                if np.abs(b).sum() - np.abs(b + 2).sum() > 0 and szA < capA:
                    inA[v] = True
                    bal[ds] += 2
                    szA += 1
                    flips += 1
        if flips == 0:
            break
    return inA


def _prep_graph(src, dst, N):
    """Relabel + shard + pad the graph. Returns per-core index arrays and the
    compile-time tile degree structure (shared by all cores)."""
    NP = ((N + C * P - 1) // (C * P)) * P            # local slots per core
    NTH = (C // 2) * NP                              # rows per table half
    assert NTH <= 32767, NTH
    TILES = NP // P
    SENT = NP - 1                                    # local sentinel slot

    inA = _balanced_split(src, dst, N, capA=NTH)
    deg = np.bincount(dst, minlength=N)
    dAn = np.bincount(dst, weights=inA[src].astype(np.float64),
                      minlength=N).astype(np.int64)
    dBn = deg - dAn

    mx = np.maximum(dAn, dBn)
    idsA = np.nonzero(inA)[0]
    idsB = np.nonzero(~inA)[0]
    idsA = idsA[np.lexsort((-dBn[idsA], -dAn[idsA], -mx[idsA]))]
    idsB = idsB[np.lexsort((-dBn[idsB], -dAn[idsB], -mx[idsB]))]
    Ch = C // 2
    order = np.full(C * NP, -1, dtype=np.int64)      # final rank -> old id
    iA = np.arange(len(idsA))
    order[(iA // Ch) * C + (iA % Ch)] = idsA         # A-positions: r%C < C/2
    iB = np.arange(len(idsB))
    order[(iB // Ch) * C + Ch + (iB % Ch)] = idsB
    newidx = np.empty(N, dtype=np.int64)             # old id -> final rank
    real = order >= 0
    pad_positions = np.nonzero(~real)[0]
    assert pad_positions.min() >= (TILES - 1) * P * C, (
        "padding spans multiple tiles; unsupported")
    newidx[order[real]] = np.nonzero(real)[0]

    r = newidx
    core_of = (r % C).astype(np.int64)
    slot_of = (r // C).astype(np.int64)
    glob_of = core_of * NP + slot_of                 # row in AG'd table

    rd = newidx[dst]
    e_core = (rd % C).astype(np.int64)
    e_slot = (rd // C).astype(np.int64)
    gsrc = glob_of[src]

    half = (gsrc >= NTH).astype(np.int64)
    key = ((e_core * NP + e_slot) * 2 + half)
    perm = np.argsort(key, kind="stable")
    gsrc_s = gsrc[perm]
    cnt = np.bincount(key[perm], minlength=C * NP * 2).reshape(C, NP, 2)
    dA_t = cnt[:, :, 0].reshape(C, TILES, P).max(axis=(0, 2))    # [TILES]
    dB_t = cnt[:, :, 1].reshape(C, TILES, P).max(axis=(0, 2))

    starts = np.zeros(C * NP * 2 + 1, dtype=np.int64)
    np.cumsum(cnt.reshape(-1), out=starts[1:])

    assert np.all(dA_t + dB_t > 0), "tile with no edges unsupported"
    per_core = []
    for c in range(C):
        cols = []
        for t in range(TILES):
            dA, dB = int(dA_t[t]), int(dB_t[t])
            a = np.full((P, dA), SENT, dtype=np.int64)
            b = np.full((P, dB), SENT, dtype=np.int64)
            base = (c * NP + t * P)
            for p in range(P):
                k = (base + p) * 2
                s0, s1 = starts[k], starts[k + 1]
                a[p, : s1 - s0] = gsrc_s[s0:s1]
                s0, s1 = starts[k + 1], starts[k + 2]
                b[p, : s1 - s0] = gsrc_s[s0:s1] - NTH
            cols.append((a.astype(np.int16), b.astype(np.int16)))
        per_core.append(cols)

    # tile groups: pair highest-degree with lowest-degree tile
    groups = []
    lo, hi = 0, TILES - 1
    while lo < hi:
        groups.append((lo, hi))
        lo += 1
        hi -= 1
    if lo == hi:
        groups.append((lo,))

    def wrap(flat):          # [n] -> [128, n//16]; ucode reads column-major over 16 partitions
        a = flat.reshape(-1, 16).T
        return np.tile(a, (8, 1)).astype(np.int16)

    # concatenated wrapped idx per core, GROUP-ordered:
    # per group: [A of t0 | A of t1 | B of t0 | B of t1]
    idx_inputs = []
    for c in range(C):
        segs = []
        for g in groups:
            aa = np.concatenate([per_core[c][t][0].T.reshape(-1) for t in g])
            bb = np.concatenate([per_core[c][t][1].T.reshape(-1) for t in g])
            if len(aa):
                segs.append(wrap(aa))
            if len(bb):
                segs.append(wrap(bb))
        idx_inputs.append(np.concatenate(segs, axis=1) if segs else
                          np.zeros((P, 0), np.int16))

    return dict(NP=NP, NTH=NTH, TILES=TILES, SENT=SENT, order=order,
                newidx=newidx, dA_t=dA_t.astype(int), dB_t=dB_t.astype(int),
                groups=groups, idx_inputs=idx_inputs)


# ------------------------------------------------------------- kernel builder
def _build(cfg):
    import concourse.bacc as bacc
    import concourse.mybir as mybir
    import concourse.tile as tile
    from concourse import bass
    from concourse.masks import make_identity

    NP, TILES = cfg["NP"], cfg["TILES"]
    dA_t, dB_t = cfg["dA_t"], cfg["dB_t"]
    groups = cfg["groups"]
    layers = cfg["layers"]          # list of dicts: Fin, Fout, HH, DD, R, relu
    IDXCOLS = cfg["IDXCOLS"]
    f32, bf16, i16 = mybir.dt.float32, mybir.dt.float16, mybir.dt.int16
    AF = mybir.ActivationFunctionType

    nc = bacc.Bacc("TRN2", target_bir_lowering=False, debug=False,
                   num_devices=C, num_swdge_queues=4,
                   dynamic_dma_scratch_size=cfg.get("SCR", 32768))

    hT0 = nc.dram_tensor("hT0", [layers[0]["Fin"], NP], f32, kind="ExternalInput")
    idx_in = nc.dram_tensor("idx_in", [P, IDXCOLS], i16, kind="ExternalInput")
    mask_in = nc.dram_tensor("mask_in", [P, 4], f32, kind="ExternalInput")
    Ws, bs = [], []
    for li, L in enumerate(layers):
        FoutE = L["Fout"] + 2 * L["HH"]     # [W | W@al | W@ar]
        Ws.append(nc.dram_tensor(f"W{li}", [L["Fin"], FoutE], f32,
                                 kind="ExternalInput"))
        bs.append(nc.dram_tensor(f"b{li}", [P, L["Fout"]], f32,
                                 kind="ExternalInput"))
    OUTF = layers[-1]["Fout"]
    y_out = nc.dram_tensor("y_out", [NP, OUTF], f32, kind="ExternalOutput")

    NL = len(layers)

    with tile.TileContext(nc) as tc:
        with (
            tc.tile_pool(name="const", bufs=1) as cp,
            tc.tile_pool(name="stageA", bufs=4) as sa,
            tc.tile_pool(name="gpool", bufs=4) as gp,
            tc.tile_pool(name="lpool", bufs=8) as lp,
            tc.tile_pool(name="opool", bufs=6) as op,
            tc.tile_pool(name="psA", bufs=2, space="PSUM") as psA,
            tc.tile_pool(name="psT", bufs=2, space="PSUM") as psT,
            tc.tile_pool(name="psO", bufs=4, space="PSUM") as psO,
            tc.tile_pool(name="dram", bufs=1, space="DRAM") as dr,
        ):
            ident = cp.tile([P, P], f32)
            make_identity(nc, ident[:])
            ident16 = cp.tile([P, P], bf16)
            nc.scalar.copy(out=ident16[:], in_=ident[:])
            mask_sb = cp.tile([P, 4], f32)
            nc.sync.dma_start(out=mask_sb[:], in_=mask_in[:])
            idx_sb = cp.tile([P, IDXCOLS], i16)
            nc.sync.dma_start(out=idx_sb[:], in_=idx_in[:])
            eps_sb = cp.tile([P, 4], f32)
            nc.scalar.activation(out=eps_sb[:], in_=mask_sb[:],
                                 func=AF.Copy, scale=0.0, bias=1e-30)

            W_sb, b_sb, er_all = [], [], []
            for li, L in enumerate(layers):
                FoutE = L["Fout"] + 2 * L["HH"]
                KC = L["Fin"] // P
                w = cp.tile([P, KC * FoutE], f32, name=f"w{li}")
                for k in range(KC):
                    nc.sync.dma_start(out=w[:, k * FoutE:(k + 1) * FoutE],
                                      in_=Ws[li][k * P:(k + 1) * P, :])
                W_sb.append(w)
                b = cp.tile([P, L["Fout"]], f32, name=f"b{li}s")
                nc.sync.dma_start(out=b[:], in_=bs[li][:])
                b_sb.append(b)
                er_all.append(cp.tile([P, TILES * L["HH"]], f32, name=f"er{li}"))

            gin = [dr.tile([NP, L["R"]], bf16, name=f"gin{li}")
                   for li, L in enumerate(layers)]
            gall = [dr.tile([C * NP, L["R"]], bf16, name=f"gall{li}",
                            addr_space="Shared")
                    for li, L in enumerate(layers)]

            def emit_table_row(li, t, pf):
                """From PSUM pf [P, Fout+2HH] of layer li, write gather-table
                row tile t of gin[li] (+ er_all[li])."""
                L = layers[li]
                Fout, HH = L["Fout"], L["HH"]
                R = L["R"]
                elo = Fout // 2                      # el offset in f32 view
                st = op.tile([P, R], bf16, name=f"st{li}_{t}", tag="st")
                nc.scalar.copy(out=st[:, :Fout], in_=pf[:, :Fout])
                if t == TILES - 1:
                    el_t = lp.tile([P, HH], f32, name=f"elm{li}_{t}", tag="elm")
                    nc.vector.tensor_tensor(out=el_t[:],
                                            in0=pf[:, Fout:Fout + HH],
                                            in1=mask_sb[:, :HH],
                                            op=mybir.AluOpType.add)
                    nc.scalar.copy(
                        out=st[:].bitcast(f32)[:, elo:elo + HH], in_=el_t[:])
                else:
                    nc.scalar.copy(
                        out=st[:].bitcast(f32)[:, elo:elo + HH],
                        in_=pf[:, Fout:Fout + HH])
                nc.scalar.copy(
                    out=er_all[li][:, t * HH:(t + 1) * HH],
                    in_=pf[:, Fout + HH:Fout + 2 * HH])
                nc.sync.dma_start(out=gin[li][t * P:(t + 1) * P, :], in_=st[:])

            # ---- stage A for layer 0 (reads hT0 from DRAM)
            L0 = layers[0]
            FoutE0 = L0["Fout"] + 2 * L0["HH"]
            KC0 = L0["Fin"] // P
            for t in range(TILES):
                hT_sb = sa.tile([P, KC0 * P], f32, name=f"h0_{t}", tag="hTl")
                for k in range(KC0):
                    nc.sync.dma_start(
                        out=hT_sb[:, k * P:(k + 1) * P],
                        in_=hT0[k * P:(k + 1) * P, t * P:(t + 1) * P])
                pf = psA.tile([P, FoutE0], f32, space="PSUM",
                              name=f"pf0_{t}", tag="pf")
                for k in range(KC0):
                    nc.tensor.matmul(out=pf[:], lhsT=hT_sb[:, k * P:(k + 1) * P],
                                     rhs=W_sb[0][:, k * FoutE0:(k + 1) * FoutE0],
                                     start=(k == 0), stop=(k == KC0 - 1))
                emit_table_row(0, t, pf)

            # ---- per layer: AllGather then stage C (fused with next stage A)
            for li, L in enumerate(layers):
                Fout, HH, DD, R = L["Fout"], L["HH"], L["DD"], L["R"]
                Rf = R // 2
                elo = Fout // 2
                NTH_l = (C // 2) * NP
                idx_off = 0

                nc.gpsimd.collective_compute(
                    "AllGather", mybir.AluOpType.bypass,
                    replica_groups=[list(range(C))],
                    ins=[gin[li][:]], outs=[gall[li][:]])

                def phase2(t, po, m_sb):
                    """o-chain + fused next-layer stage A + table emit.
                    Emitted LAG tiles behind phase1 so tile t's late ops
                    (waiting on the PE matmul chain) don't block tile t+1's
                    early DVE/ACT ops in the in-order engine streams."""
                    o_sb = op.tile([P, Fout], f32, name=f"o{li}_{t}", tag="o")
                    r3 = m_sb[:, HH:2 * HH].unsqueeze(2) \
                        .to_broadcast([P, HH, DD])
                    nc.vector.tensor_tensor(
                        out=o_sb[:].rearrange("p (h f) -> p h f", h=HH),
                        in0=po[:].rearrange("p (h f) -> p h f", h=HH),
                        in1=r3, op=mybir.AluOpType.mult)
                    nc.vector.tensor_tensor(out=o_sb[:], in0=o_sb[:],
                                            in1=b_sb[li][:],
                                            op=mybir.AluOpType.add)
                    if L["relu"]:
                        nc.scalar.activation(out=o_sb[:], in_=o_sb[:],
                                             func=AF.Relu)
                    if li + 1 < NL:
                        Ln = layers[li + 1]
                        FoutEn = Ln["Fout"] + 2 * Ln["HH"]
                        KCn = Ln["Fin"] // P
                        pf = psA.tile([P, FoutEn], f32, space="PSUM",
                                      name=f"pf{li + 1}_{t}", tag="pf")
                        for k in range(KCn):
                            pt = psT.tile([P, P], f32, space="PSUM",
                                          name=f"pt{li}_{t}_{k}", tag="pt")
                            nc.tensor.transpose(
                                out=pt[:], in_=o_sb[:, k * P:(k + 1) * P],
                                identity=ident[:])
                            tt = op.tile([P, P], f32,
                                         name=f"tt{li}_{t}_{k}", tag="tt")
                            nc.scalar.copy(out=tt[:], in_=pt[:])
                            nc.tensor.matmul(
                                out=pf[:], lhsT=tt[:],
                                rhs=W_sb[li + 1][:, k * FoutEn:(k + 1) * FoutEn],
                                start=(k == 0), stop=(k == KCn - 1))
                        emit_table_row(li + 1, t, pf)
                    else:
                        nc.sync.dma_start(out=y_out[t * P:(t + 1) * P, :],
                                          in_=o_sb[:])

                LAG = 2
                pend = []
                for gi, g in enumerate(groups):
                    dAs = [int(dA_t[t]) for t in g]
                    dBs = [int(dB_t[t]) for t in g]
                    GA, GB = sum(dAs), sum(dBs)
                    DEGG = GA + GB
                    G = gp.tile([P, DEGG * R], bf16, name=f"G{li}_{gi}", tag="G")
                    if GA:
                        na = 8 * GA
                        nc.gpsimd.dma_gather(
                            G[:, :GA * R].rearrange("p (d r) -> p d r", d=GA),
                            gall[li][0:NTH_l, :], idx_sb[:, idx_off:idx_off + na],
                            P * GA, P * GA, R,
                            single_packet=False, queue_num=gi % 4)
                        idx_off += na
                    if GB:
                        nb = 8 * GB
                        nc.gpsimd.dma_gather(
                            G[:, GA * R:].rearrange("p (d r) -> p d r", d=GB),
                            gall[li][NTH_l:2 * NTH_l, :],
                            idx_sb[:, idx_off:idx_off + nb],
                            P * GB, P * GB, R,
                            single_packet=False, queue_num=(gi + 2) % 4)
                        idx_off += nb

                    offs = {}
                    ao = 0
                    for t, dA in zip(g, dAs):
                        offs[t] = [(ao, dA)]
                        ao += dA
                    for t, dB in zip(g, dBs):
                        offs[t].append((ao, dB))
                        ao += dB

                    for t in g:
                        (oA, dA), (oB, dB) = offs[t]
                        DEG = dA + dB
                        l_sb = lp.tile([P, DEG * HH], f32, name=f"l{li}_{t}",
                                       tag="l")
                        e_sb = lp.tile([P, DEG * HH], f32, name=f"e{li}_{t}",
                                       tag="e")
                        m_sb = lp.tile([P, 2 * HH], f32, name=f"m{li}_{t}",
                                       tag="m")
                        Gf = G[:].bitcast(f32)
                        er3 = er_all[li][:, t * HH:(t + 1) * HH]
                        # l = el[src] + er[dst]  (d-major [P, DEG, HH])
                        for (o0, dd, lo) in ((oA, dA, 0), (oB, dB, dA)):
                            if dd == 0:
                                continue
                            el3 = Gf[:, o0 * Rf:(o0 + dd) * Rf] \
                                .rearrange("p (d r) -> p d r", d=dd) \
                                [:, :, elo:elo + HH]
                            nc.vector.tensor_tensor(
                                out=l_sb[:, lo * HH:(lo + dd) * HH]
                                .rearrange("p (d h) -> p d h", h=HH),
                                in0=el3,
                                in1=er3.unsqueeze(1).to_broadcast([P, dd, HH]),
                                op=mybir.AluOpType.add)
                        # leaky relu on ACT (Prelu consumes alpha; Lrelu is
                        # fixed slope 0.01), then exp + per-head sums
                        nc.scalar.activation(out=l_sb[:], in_=l_sb[:],
                                             func=AF.Prelu, alpha=0.2)
                        for h in range(HH):
                            nc.scalar.activation(
                                out=e_sb[:, h::HH], in_=l_sb[:, h::HH],
                                func=AF.Exp, accum_out=m_sb[:, h:h + 1])
                        # clamp s away from 0 (all-sentinel pad rows) + recip
                        # (tensor_tensor never grabs the GpSimd-shared port)
                        nc.vector.tensor_tensor(
                            out=m_sb[:, :HH], in0=m_sb[:, :HH],
                            in1=eps_sb[:, :HH], op=mybir.AluOpType.max)
                        nc.vector.reciprocal(out=m_sb[:, HH:2 * HH],
                                             in_=m_sb[:, :HH])
                        # weighted multiply in-place on G feat region
                        for (o0, dd, lo) in ((oA, dA, 0), (oB, dB, dA)):
                            if dd == 0:
                                continue
                            g4 = G[:, o0 * R:(o0 + dd) * R] \
                                .rearrange("p (d r) -> p d r", d=dd)[:, :, :Fout] \
                                .rearrange("p d (h f) -> p d h f", h=HH)
                            e4 = e_sb[:, lo * HH:(lo + dd) * HH] \
                                .rearrange("p (d h) -> p d h", h=HH) \
                                .unsqueeze(3).to_broadcast([P, dd, HH, DD])
                            nc.vector.tensor_tensor(out=g4, in0=g4, in1=e4,
                                                    op=mybir.AluOpType.mult)
                        # accumulate over deg on TensorE (identity matmul)
                        po = psO.tile([P, Fout], f32, space="PSUM",
                                      name=f"po{li}_{t}", tag="po")
                        mm = 0
                        for (o0, dd) in ((oA, dA), (oB, dB)):
                            for d in range(dd):
                                nc.tensor.matmul(
                                    out=po[:], lhsT=ident16[:],
                                    rhs=G[:, (o0 + d) * R:(o0 + d) * R + Fout],
                                    start=(mm == 0), stop=(mm == DEG - 1))
                                mm += 1
                        pend.append((t, po, m_sb))
                        if len(pend) > LAG:
                            phase2(*pend.pop(0))
                while pend:
                    phase2(*pend.pop(0))
    nc.compile()
    return nc


# ---------------------------------------------------------------- entrypoint
_CACHE = {}


def kernel(features, src, dst, W1, al1, ar1, b1, W2, al2, ar2, b2,
           W3, al3, ar3, b3):
    import jax
    jax.config.update("jax_compilation_cache_dir", "/tmp/jaxcache")
    jax.config.update("jax_persistent_cache_min_compile_time_secs", 0.0)
    jax.config.update("jax_persistent_cache_min_entry_size_bytes", 0)
    from concourse.bass_utils import run_bass_kernel_spmd

    features = np.asarray(features, dtype=np.float32)
    src = np.asarray(src).astype(np.int64)
    dst = np.asarray(dst).astype(np.int64)
    N, IN = features.shape
    H, HID = np.asarray(al1).shape
    OUT = np.asarray(W3).shape[1]
    H3 = np.asarray(al3).shape[0]
    assert H3 == 1, "layer-3 head-mean only implemented for H3=1"

    g = _prep_graph(src, dst, N)
    NP, TILES = g["NP"], g["TILES"]

    def rnd(fout, hh):  # table row elems (bf16): feat + 2*HH el-f32 -> mult of 128
        return ((fout + 2 * hh + 127) // 128) * 128

    layers = [
        dict(Fin=IN, Fout=H * HID, HH=H, DD=HID, R=rnd(H * HID, H), relu=True),
        dict(Fin=H * HID, Fout=H * HID, HH=H, DD=HID, R=rnd(H * HID, H), relu=True),
        dict(Fin=H * HID, Fout=OUT, HH=1, DD=OUT, R=rnd(OUT, 1), relu=False),
    ]
    IDXCOLS = sum(8 * (int(a) + int(b)) for a, b in zip(g["dA_t"], g["dB_t"]))

    key = (N, len(src), IN, H, HID, OUT, tuple(g["dA_t"]), tuple(g["dB_t"]))
    if key not in _CACHE:
        cfg = dict(NP=NP, TILES=TILES, dA_t=g["dA_t"], dB_t=g["dB_t"],
                   groups=g["groups"], layers=layers, IDXCOLS=IDXCOLS)
        _CACHE[key] = _build(cfg)
    nc = _CACHE[key]

    def wext(W, al, ar, hh, dd):
        W = np.asarray(W, np.float64)
        al = np.asarray(al, np.float64).reshape(hh, dd)
        ar = np.asarray(ar, np.float64).reshape(hh, dd)
        Wh = W.reshape(W.shape[0], hh, dd)
        wal = np.einsum("fhd,hd->fh", Wh, al)
        war = np.einsum("fhd,hd->fh", Wh, ar)
        return np.concatenate([W, wal, war], axis=1).astype(np.float32)

    W1e = wext(W1, al1, ar1, H, HID)
    W2e = wext(W2, al2, ar2, H, HID)
    W3e = wext(W3, al3, ar3, 1, OUT)

    order = g["order"]
    assert C * NP - N < C * P, "padding spans multiple tiles; unsupported"
    ins = []
    rep = lambda v: np.repeat(np.asarray(v, np.float32).reshape(1, -1), P, axis=0)
    for c in range(C):
        ranks = np.arange(NP) * C + c
        valid = (order[ranks] >= 0)
        h0 = np.zeros((NP, IN), np.float32)
        h0[valid] = features[order[ranks[valid]]]
        d = {
            "hT0": np.ascontiguousarray(h0.T),
            "idx_in": g["idx_inputs"][c],
            "W0": W1e, "W1": W2e, "W2": W3e,
            "b0": rep(np.asarray(b1).reshape(-1)),
            "b1": rep(np.asarray(b2).reshape(-1)),
            "b2": rep(np.asarray(b3).reshape(-1)),
        }
        mk = np.zeros((P, 4), np.float32)
        padrows = np.nonzero(~valid[(TILES - 1) * P:])[0]
        mk[padrows, :] = -1e30
        d["mask_in"] = mk
        ins.append(d)

    runkw = {}
    if os.environ.get("GAT_TRACE") == "1":
        try:
            import ntff_hook
            ntff_hook.install()
            runkw["trace"] = True
        except Exception:
            pass
    res = run_bass_kernel_spmd(nc, ins, core_ids=list(range(C)), **runkw)
    out = np.zeros((N, OUT), np.float32)
    for c in range(C):
        ranks = np.arange(NP) * C + c
        valid = (order[ranks] >= 0)
        out[order[ranks[valid]]] = res.results[c]["y_out"][valid]
    kernel.last_results = res
    return out
